# revision 1
# baseline (speedup 1.0000x reference)
"""Block-causal multi-head attention (B=1, S=4096, E=1024, H=16, BLK=128) on 8 trn2 cores.

Strategy (head-parallel attention + sequence-parallel out_proj), all big matmuls
as fp8e4m3 DoubleRow (0.5 cycles/row) with residual-pair error correction:

  - Host: splits x, in_proj, out_proj into fp8 value+residual pairs laid out as
    [chunk, 128, 2, n] so contraction pairs map onto DoubleRow k-tiles.
  - QKV proj per core (2 heads): 3-term product
    (x8+dx8)(w8+dw8) ~= x8 w8 + dx8 w8 + x8 dw8 — 12 DoubleRow matmuls per
    [128, 512] psum instead of 8 fp32r matmuls (3072 vs 4096 cycles), error ~0.1%.
  - Scores^T [128k x 512q] per (key-block, q-group): lhsT carries (k8, k8) and
    rhs carries (q8, dq8) so s = k8^T (q8 + dq8); only k8's ~2.5% rounding
    survives (end-to-end rel err ~1.1e-2 vs the 2e-2 budget).
  - exp(0.125 s) on ACT writes fp16 P; PV in fp16 with V augmented with 64 ones
    columns so every acc psum partition 64..127 holds the softmax denominator;
    normalize is then pure DVE: reciprocal of acc[64:128] times acc[0:64]
    (no PE broadcast needed).
  - Normalized attn^T is split into fp8 value+residual, AllToAll'd (fp8), and
    out_proj runs the same 3-term DoubleRow scheme; host concatenates y^T.
"""
import numpy as np
import ml_dtypes

import concourse.bass as bass
import concourse.mybir as mybir
from concourse import bacc, tile
from concourse.bass_utils import run_bass_kernel_spmd
from concourse.masks import make_identity

N_CORES = 8
S, E, H, BLK, D = 4096, 1024, 16, 128, 64
NB = S // BLK            # 32 key/query blocks
NG = 8                   # q-groups of 512
GQ = 512                 # q columns per group
HPC = H // N_CORES       # heads per core (2)
RPC = 3 * HPC * D        # in_proj rows per core (384)

F32 = mybir.dt.float32
F32R = mybir.dt.float32r
F16 = mybir.dt.float16
FP8 = mybir.dt.float8e4
ALU = mybir.AluOpType
ACTF = mybir.ActivationFunctionType
PM = mybir.MatmulPerfMode
E4M3 = ml_dtypes.float8_e4m3


def build_nc(reps: int = 1, cc: bool = True):
    nc = bacc.Bacc("TRN2", target_bir_lowering=False, debug=False, num_devices=N_CORES)

    x8T = nc.dram_tensor("x8T", [4, 128, 2, S], FP8, kind="ExternalInput")
    dx8T = nc.dram_tensor("dx8T", [4, 128, 2, S], FP8, kind="ExternalInput")
    wq8T = nc.dram_tensor("wq8T", [4, 128, 2, RPC], FP8, kind="ExternalInput")
    dwq8T = nc.dram_tensor("dwq8T", [4, 128, 2, RPC], FP8, kind="ExternalInput")
    bqkv = nc.dram_tensor("bqkv", [3, 2 * D], F32, kind="ExternalInput")
    wo8T = nc.dram_tensor("wo8T", [4, 128, 2, E], FP8, kind="ExternalInput")
    dwo8T = nc.dram_tensor("dwo8T", [4, 128, 2, E], FP8, kind="ExternalInput")
    bout = nc.dram_tensor("bout", [8, 128], F32, kind="ExternalInput")
    yT = nc.dram_tensor("yT", [E, GQ], F32, kind="ExternalOutput")

    with tile.TileContext(nc) as tc:
        with (
            tc.tile_pool(name="const", bufs=1) as constp,
            tc.tile_pool(name="qkv", bufs=1) as qkvp,
            tc.tile_pool(name="xt", bufs=16) as xtp,
            tc.tile_pool(name="pt", bufs=4) as ptp,
            tc.tile_pool(name="vst", bufs=2) as vstp,
            tc.tile_pool(name="small", bufs=4) as smallp,
            tc.tile_pool(name="attn", bufs=4) as attnp,
            tc.tile_pool(name="ytp", bufs=2) as ytp,
            tc.tile_pool(name="pp", bufs=2, space="PSUM") as pp,
            tc.tile_pool(name="scores", bufs=2, space="PSUM") as scp,
            tc.tile_pool(name="accum", bufs=2, space="PSUM") as accp,
            tc.tile_pool(name="dram", bufs=1, space="DRAM") as dram,
        ):
            # ---- constants / weights ----
            ident16 = constp.tile([128, 128], F16)
            make_identity(nc, ident16[:])
            bq_sb = constp.tile([128, 3], F32)
            bo_sb = constp.tile([128, 8], F32)
            wq8_sb = constp.tile([128, 4 * 2 * RPC], FP8)
            dwq8_sb = constp.tile([128, 4 * 2 * RPC], FP8)
            wq8v = wq8_sb[:].rearrange("p (c t r) -> p c t r", c=4, t=2)
            dwq8v = dwq8_sb[:].rearrange("p (c t r) -> p c t r", c=4, t=2)

            def load_wq(t):
                dst, src = (wq8v, wq8T) if t < 4 else (dwq8v, dwq8T)
                c = t % 4
                nc.sync.dma_start(dst[:, c, :, :], src.ap()[c])

            def load_biases():
                nc.sync.dma_start(bq_sb[:], bqkv.ap().rearrange("r p -> p r"))
                nc.sync.dma_start(bo_sb[:], bout.ap().rearrange("t p -> p t"))

            wo8_sb = constp.tile([128, 4 * 2 * E], FP8)
            dwo8_sb = constp.tile([128, 4 * 2 * E], FP8)
            wo8v = wo8_sb[:].rearrange("p (m t e) -> p m t e", m=4, t=2)
            dwo8v = dwo8_sb[:].rearrange("p (m t e) -> p m t e", m=4, t=2)

            def load_wout():
                for c in range(4):
                    nc.sync.dma_start(wo8v[:, c, :, :], wo8T.ap()[c])
                    nc.sync.dma_start(dwo8v[:, c, :, :], dwo8T.ap()[c])

            # persistent per-rep tensors; partitions 0:64 head0, 64:128 head1.
            # qt8 t-slots: (q8, dq8) residual pair; kt8 t-slots: (k8, k8).
            qt8 = qkvp.tile([128, 2, S], FP8, tag="qt")
            kt8 = qkvp.tile([128, 2, S], FP8, tag="kt")
            v_sb = qkvp.tile([128, 2 * NB * 2 * D], F16, tag="vsb")
            v_view = v_sb[:].rearrange("p (h b dd) -> p h b dd", h=2, b=NB)
            cc_in = dram.tile([N_CORES, 128, 2 * GQ], FP8, tag="ccin")
            cc_out = dram.tile([N_CORES, 128, 2 * GQ], FP8, tag="ccout")

            for rep in range(reps):
                # ones columns of V (denominator trick); rewritten each rep
                nc.vector.memset(v_view[:, :, :, D:2 * D], 1.0)

                # ---------- proj work-item machinery ----------
                def xt_dmas(g):
                    xts, dxts = [], []
                    for t in range(8):
                        if g == 0 and rep == 0:
                            load_wq(t)      # interleave weight chunks with first x tiles
                        src = x8T if t < 4 else dx8T
                        c = t % 4
                        xt = xtp.tile([128, 2, GQ], FP8, tag="xt")
                        nc.sync.dma_start(
                            xt[:], src.ap()[c][:, :, g * GQ:(g + 1) * GQ])
                        (xts if t < 4 else dxts).append(xt)
                    if g == 0 and rep == 0:
                        load_biases()
                    return xts, dxts

                def proj_items(g, xtpair):
                    """Yield closures emitting proj instructions for group g."""
                    xts, dxts = xtpair
                    sl = slice(g * GQ, (g + 1) * GQ)

                    def rtile(which):
                        ps = pp.tile([128, GQ], F32, tag="pp")
                        msl = slice(which * 128, (which + 1) * 128)
                        terms = [(wq8v, xts), (dwq8v, xts), (wq8v, dxts)]
                        i = 0
                        for wv, xtiles in terms:
                            for c in range(4):
                                yield lambda wv=wv, c=c, xt=xtiles[c], i=i, ps=ps: \
                                    nc.tensor.matmul(
                                        ps[:], wv[:, c, :, msl], xt[:],
                                        start=(i == 0), stop=(i == 11),
                                        perf_mode=PM.DoubleRow)
                                i += 1
                        if which == 0:      # q: q8 + residual dq8 (scale folded into exp)
                            qtmp = smallp.tile([128, GQ], F32, tag="qtmp")
                            yield lambda ps=ps, qtmp=qtmp: nc.vector.tensor_scalar(
                                qtmp[:], ps[:], bq_sb[:, 0:1], None, ALU.add)
                            yield lambda qtmp=qtmp: nc.vector.tensor_copy(
                                qt8[:, 0, sl], qtmp[:])
                            yield lambda qtmp=qtmp: nc.vector.tensor_tensor(
                                qt8[:, 1, sl], qtmp[:], qt8[:, 0, sl], ALU.subtract)
                        elif which == 1:    # k: k8 duplicated into both t-slots
                            yield lambda ps=ps: nc.vector.tensor_scalar(
                                kt8[:, 0, sl], ps[:], bq_sb[:, 1:2], None, ALU.add)
                            yield lambda: nc.vector.tensor_copy(
                                kt8[:, 1, sl], kt8[:, 0, sl])
                        else:               # v^T staging: psum + bv (fp16)
                            vt = vstp.tile([128, GQ], F16, tag="vst")
                            yield lambda ps=ps, vt=vt: nc.vector.tensor_scalar(
                                vt[:], ps[:], bq_sb[:, 2:3], None, ALU.add)
                            for j in range(4):
                                bk = 4 * g + j

                                def tr(j=j, bk=bk, vt=vt):
                                    trp = pp.tile([128, 128], F16, tag="pp")
                                    nc.tensor.transpose(
                                        trp[0:128, 0:128], vt[:, j * 128:(j + 1) * 128],
                                        ident16[:])
                                    nc.vector.tensor_copy(
                                        v_view[:, :, bk, 0:D],
                                        trp[0:128, 0:128].rearrange("p (h d) -> p h d", h=2))
                                yield tr
                    yield from rtile(0)
                    yield from rtile(1)
                    yield from rtile(2)

                def attention_group(g, pending):
                    """Emit attention for q-group g, interleaving `pending` proj items."""
                    nbk = 4 * g + 4
                    # throttle interleaved proj items in the first two blocks so the
                    # group's exp pipeline primes before PE picks up filler work
                    quota = []
                    rem = len(pending)
                    for i in range(nbk):
                        if i < 2:
                            q = min(rem, 1)
                        else:
                            left = nbk - i
                            q = (rem + left - 1) // left
                        quota.append(q)
                        rem -= q
                    pt_tiles = {}
                    acc_a = accp.tile([128, GQ], F32, tag="acc")
                    acc_b = accp.tile([128, GQ], F32, tag="acc")
                    for bk in range(nbk):
                        qoff = max(0, (bk - 4 * g)) * 128
                        sc = scp.tile([128, 2 * GQ], F32, tag="sc")
                        ksl = slice(bk * 128, (bk + 1) * 128)
                        qsl = slice(g * GQ + qoff, (g + 1) * GQ)
                        nc.tensor.matmul(
                            sc[:, qoff:GQ],
                            kt8[0:64, :, ksl], qt8[0:64, :, qsl],
                            start=True, stop=True, skip_group_check=True,
                            perf_mode=PM.DoubleRow)
                        nc.tensor.matmul(
                            sc[:, GQ + qoff:2 * GQ],
                            kt8[64:128, :, ksl], qt8[64:128, :, qsl],
                            start=True, stop=True, skip_group_check=True,
                            perf_mode=PM.DoubleRow)
                        pt = ptp.tile([128, 2 * GQ], F16, tag="pt")
                        # q,k carry the host-side LAM=32 weight scale; fold
                        # 1/LAM^2 into the exp scale.
                        ESC = 0.125 / 1024.0
                        if qoff == 0:
                            nc.scalar.activation(pt[:, 0:2 * GQ], sc[:, 0:2 * GQ],
                                                 ACTF.Exp, scale=ESC)
                        else:   # skip the unwritten [GQ:GQ+qoff] gap between heads
                            nc.scalar.activation(pt[:, qoff:GQ], sc[:, qoff:GQ],
                                                 ACTF.Exp, scale=ESC)
                            nc.scalar.activation(pt[:, GQ + qoff:2 * GQ],
                                                 sc[:, GQ + qoff:2 * GQ],
                                                 ACTF.Exp, scale=ESC)
                        pt_tiles[bk] = (pt, qoff)
                        # PV for the previous block (keeps PE busy while ACT exps)
                        if bk > 0:
                            emit_pv(g, bk - 1, pt_tiles, acc_a, acc_b)
                        for _ in range(quota[bk]):
                            if pending:
                                pending.pop(0)()
                    emit_pv(g, nbk - 1, pt_tiles, acc_a, acc_b, last=True)
                    while pending:
                        pending.pop(0)()
                    return normalize_items(g, acc_a, acc_b)

                def normalize_items(g, acc_a, acc_b):
                    # deferred normalize + all-to-all staging closures for group g.
                    # acc rows 64:128 all hold the softmax denominator (64 ones
                    # columns in V), so normalize is partition-aligned DVE work.
                    items = []
                    for h, acc in ((0, acc_a), (1, acc_b)):
                        def norm(h=h, acc=acc, g=g):
                            recip = smallp.tile([64, GQ], F32R, tag="recip")
                            with nc.allow_low_precision(reason="softmax denom reciprocal in fp32r"):
                                nc.vector.reciprocal(recip[:], acc[64:128, :])
                            outf = smallp.tile([64, GQ], F32, tag="outf")
                            nc.vector.tensor_tensor(outf[:], acc[0:64, :], recip[:], ALU.mult)
                            at8s = smallp.tile([64, GQ], FP8, tag="at8s")
                            nc.vector.tensor_copy(at8s[:], outf[:])
                            dat8s = smallp.tile([64, GQ], FP8, tag="dat8s")
                            nc.vector.tensor_tensor(dat8s[:], outf[:], at8s[:], ALU.subtract)
                            nc.gpsimd.dma_start(cc_in[:][g, h * 64:(h + 1) * 64, 0:GQ], at8s[:])
                            nc.gpsimd.dma_start(cc_in[:][g, h * 64:(h + 1) * 64, GQ:2 * GQ], dat8s[:])
                        items.append(norm)
                    return items

                def emit_pv(g, bk, pt_tiles, acc_a, acc_b, last=False):
                    pt, qoff = pt_tiles.pop(bk)
                    nc.tensor.matmul(
                        acc_a[:, qoff:GQ], v_view[:, 0, bk, :], pt[:, qoff:GQ],
                        start=(bk == 0), stop=last, skip_group_check=True)
                    nc.tensor.matmul(
                        acc_b[:, qoff:GQ], v_view[:, 1, bk, :],
                        pt[:, GQ + qoff:2 * GQ],
                        start=(bk == 0), stop=last, skip_group_check=True)

                # ---------- emit: proj(0) then attention groups with lookahead ----------
                xtpair = xt_dmas(0)
                for item in proj_items(0, xtpair):
                    item()
                carry = []
                for g in range(NG):
                    if g + 1 < NG:
                        nxt = xt_dmas(g + 1)
                        pending = carry + list(proj_items(g + 1, nxt))
                    else:
                        pending = carry
                    carry = attention_group(g, pending)
                    if g == 1:
                        load_wout()
                for item in carry:
                    item()

                # ---------- all-to-all + out_proj ----------
                if cc:
                    nc.gpsimd.collective_compute(
                        "AllToAll", ALU.bypass,
                        replica_groups=[list(range(N_CORES))],
                        ins=[cc_in.opt()], outs=[cc_out.opt()])
                else:
                    nc.gpsimd.dma_start(cc_out[:], cc_in[:])
                at8_sb = attnp.tile([128, 4 * 2 * GQ], FP8, tag="at")
                dat8_sb = attnp.tile([128, 4 * 2 * GQ], FP8, tag="at")
                at8v = at8_sb[:].rearrange("p (m t q) -> p m t q", m=4, t=2)
                dat8v = dat8_sb[:].rearrange("p (m t q) -> p m t q", m=4, t=2)
                for j in range(N_CORES):
                    nc.gpsimd.dma_start(at8v[:, j // 2, j % 2, :], cc_out[:][j, :, 0:GQ])
                    nc.gpsimd.dma_start(dat8v[:, j // 2, j % 2, :], cc_out[:][j, :, GQ:2 * GQ])
                for t in range(8):
                    ps = pp.tile([128, GQ], F32, tag="pp")
                    tsl = slice(t * 128, (t + 1) * 128)
                    i = 0
                    for wv, atv in ((wo8v, at8v), (dwo8v, at8v), (wo8v, dat8v)):
                        for m in range(4):
                            nc.tensor.matmul(
                                ps[:], wv[:, m, :, tsl], atv[:, m, :, :],
                                start=(i == 0), stop=(i == 11),
                                perf_mode=PM.DoubleRow)
                            i += 1
                    yt = ytp.tile([128, GQ], F32, tag="yt")
                    # psum carries LAM^2 = 1024x scale (32x attn * 32x w_out);
                    # bout is shipped pre-multiplied by 1024.
                    nc.vector.tensor_scalar(yt[:], ps[:], bo_sb[:, t:t + 1],
                                            1.0 / 1024.0, ALU.add, ALU.mult)
                    nc.sync.dma_start(yT.ap()[t * 128:(t + 1) * 128, :], yt[:])

    nc.compile()
    return nc


_NC_CACHE = {}


def _get_nc(reps=1):
    if reps not in _NC_CACHE:
        _NC_CACHE[reps] = build_nc(reps)
    return _NC_CACHE[reps]


def _pair_fp8(a):
    """[1024, N] -> fp8 (value, residual) pairs shaped [4, 128, 2, N]."""
    r = np.ascontiguousarray(
        a.reshape(4, 2, 128, a.shape[1]).transpose(0, 2, 1, 3))
    a8 = r.astype(E4M3)
    da8 = (r - a8.astype(np.float32)).astype(E4M3)
    return a8, da8


def make_in_maps(x, in_proj_weight, in_proj_bias, out_proj_weight, out_proj_bias):
    x = np.asarray(x, np.float32)
    w_in = np.asarray(in_proj_weight, np.float32)
    b_in = np.asarray(in_proj_bias, np.float32)
    w_out = np.asarray(out_proj_weight, np.float32)
    b_out = np.asarray(out_proj_bias, np.float32)

    # Weights have std ~0.02 — below e4m3's min-normal 2^-6 — so their fp8
    # residuals would vanish in the subnormal floor. Scale weights by LAM=32
    # (exact power of two) and compensate in the exp scale (1/LAM^2) and the
    # final output scale (1/LAM^2).
    LAM = 32.0
    xT = np.ascontiguousarray(x.reshape(S, E).T)
    x8T, dx8T = _pair_fp8(xT)
    wo8T, dwo8T = _pair_fp8(np.ascontiguousarray(w_out.T) * LAM)
    bout = np.ascontiguousarray(b_out.reshape(8, 128)) * (LAM * LAM)
    in_maps = []
    for c in range(N_CORES):
        rows = []
        for blk in range(3):  # q, k, v blocks of in_proj
            for h in (2 * c, 2 * c + 1):
                rows.extend(range(blk * E + h * D, blk * E + (h + 1) * D))
        rows = np.array(rows)
        wq8T, dwq8T = _pair_fp8(np.ascontiguousarray(w_in[rows].T) * LAM)  # [1024, 384]
        bqkv = np.ascontiguousarray(b_in[rows].reshape(3, 2 * D)) * LAM
        in_maps.append({
            "x8T": x8T, "dx8T": dx8T, "wq8T": wq8T, "dwq8T": dwq8T,
            "bqkv": bqkv, "wo8T": wo8T, "dwo8T": dwo8T, "bout": bout,
        })
    return in_maps


def assemble_output(results):
    yT_full = np.concatenate([results[c]["yT"] for c in range(N_CORES)], axis=1)
    return np.ascontiguousarray(yT_full.T).reshape(1, S, E).astype(np.float32)


def kernel(x, in_proj_weight, in_proj_bias, out_proj_weight, out_proj_bias,
           block_size, num_heads):
    assert int(np.asarray(block_size)) == BLK and int(np.asarray(num_heads)) == H
    in_maps = make_in_maps(x, in_proj_weight, in_proj_bias,
                           out_proj_weight, out_proj_bias)
    nc = _get_nc(1)
    res = run_bass_kernel_spmd(nc, in_maps, core_ids=list(range(N_CORES)))
    return assemble_output(res.results)

